# revision 29
# baseline (speedup 1.0000x reference)
"""MoE layer (1024 -> 4096 gelu -> 1024, E=8 experts, top-2) on 8 TRN2 cores.

Strategy (expert parallelism, per the sharding hint):
  - Host computes the gating linear + top-2 routing (67 MFLOP, 0.02% of the
    total work) and builds one token batch per expert ("all-to-all" dispatch).
  - Core e holds expert e's weights and runs the FFN for the tokens routed to
    it: y = gate * (gelu(x @ W1 + b1) @ W2 + b2), with the gate weight folded
    into the device epilogue.
  - Host scatter-adds the two expert outputs per token back together
    (the reverse all-to-all / unshard step).

Device kernel (per core, SPMD over 8 cores):
  - Activations live transposed (xt = X^T) so both matmul layers use natural
    weight layouts with the contraction on the partition axis.
  - W1/W2 are streamed from HBM in 8 h-blocks of 512 (32 MB per core total,
    double buffered) so weight traffic is exactly one pass.
  - Matmuls run in float32r (full-rate fp32 path on the PE, ~1e-4 precision
    vs ~3e-3 for bf16).
  - Layer 1 writes Ht = gelu(W1_blk^T @ xt + b1) [h x C] via the ACT engine;
    layer 2 accumulates Y += Ht^T @ W2_blk in PSUM over the block's 4 k-tiles
    and folds the partial into an SBUF accumulator on the DVE engine.
  - b2 is folded into the first partial sum; the epilogue scales rows by
    the gate weight and streams the result out.
"""
import numpy as np

D_IN = 1024
D_HID = 4096
D_OUT = 1024
N_EXPERTS = 8
TOP_K = 2
N_CORES = 8

HB = 512                      # h-block size streamed per iteration
N_HB = D_HID // HB            # 8 blocks
KT = D_IN // 128              # 8 contraction tiles for layer 1
HKT = HB // 128               # 4 contraction tiles per block for layer 2
CHUNK = 384                   # layer-1 moving-dim chunk (<=512, >=256 for f32r rate)

TRACE = False                 # set by test.py to collect a HW profile
LAST_EXEC_NS = None
LAST_PROFILE = None


def _build_bass(C):
    import concourse.bacc as bacc
    import concourse.tile as tile
    import concourse.mybir as mybir

    F32 = mybir.dt.float32
    F32R = mybir.dt.float32r
    AF = mybir.ActivationFunctionType

    NT = C // 128             # token tiles
    NCH = C // CHUNK          # layer-1 chunks

    nc = bacc.Bacc("TRN2", target_bir_lowering=False, debug=False)
    xt_d = nc.dram_tensor("xt", [D_IN, C], F32R, kind="ExternalInput").ap()
    w1_d = nc.dram_tensor("w1", [D_IN, D_HID], F32R, kind="ExternalInput").ap()
    w2_d = nc.dram_tensor("w2", [D_HID, D_OUT], F32R, kind="ExternalInput").ap()
    b1_d = nc.dram_tensor("b1c", [128, D_HID // 128], F32, kind="ExternalInput").ap()
    b2_d = nc.dram_tensor("b2b", [128, D_OUT], F32, kind="ExternalInput").ap()
    g_d = nc.dram_tensor("g", [128, NT], F32, kind="ExternalInput").ap()
    y_d = nc.dram_tensor("y", [C, D_OUT], F32, kind="ExternalOutput").ap()

    with tile.TileContext(nc) as tc:
        with (
            tc.tile_pool(name="persist", bufs=1) as persist,
            tc.tile_pool(name="w1p", bufs=2) as w1p,
            tc.tile_pool(name="w2p", bufs=2) as w2p,
            tc.tile_pool(name="htp", bufs=2) as htp,
            tc.tile_pool(name="ps1", bufs=3, space="PSUM") as ps1,
            tc.tile_pool(name="ps2", bufs=2, space="PSUM") as ps2,
            tc.tile_pool(name="ps_w", bufs=1, space="PSUM") as ps_w,
        ):
            # ---- PE warmup: keep the PE busy (and HAM un-throttled) while
            # the input DMAs stream in.  Zero matmuls into a scratch bank.
            warm = persist.tile([128, 512], mybir.dt.bfloat16, tag="warm")
            nc.gpsimd.memset(warm[:], 0.0)
            wps = ps_w.tile([128, 512], F32, tag="wps")
            for _ in range(40):
                nc.tensor.matmul(wps[:], warm[:, 0:128], warm[:], start=True, stop=True)

            # ---- persistent loads ----
            # xt loads are split by layer-1 chunk so the first matmul group
            # only waits for 1/NCH of the activations (chunk 0 of every
            # k-tile), letting the PE start while the rest streams in.
            xt = []
            for k in range(KT):
                xk = persist.tile([128, C], F32R, tag=f"xt{k}")
                xt.append(xk)
            for k in range(KT):
                nc.sync.dma_start(
                    xt[k][:, 0:CHUNK], xt_d[k * 128:(k + 1) * 128, 0:CHUNK]
                )
            b1c = persist.tile([128, D_HID // 128], F32, tag="b1c")
            nc.sync.dma_start(b1c[:], b1_d[:])
            b2b = persist.tile([128, D_OUT], F32, tag="b2b")
            nc.sync.dma_start(b2b[:], b2_d[:])
            gt = persist.tile([128, NT], F32, tag="g")
            nc.sync.dma_start(gt[:], g_d[:])
            y_acc = []
            for tm in range(NT):
                y_tm = persist.tile([128, D_OUT], F32, tag=f"y{tm}")
                y_acc.append(y_tm)

            # ---- h-blocked FFN ----
            for hb in range(N_HB):
                # stream this block's weights (full-BW HWDGE, f32r-direct);
                # the first block loads in column halves so the first matmul
                # group (m=0..1) waits for half the bytes
                w1t = []
                for k in range(KT):
                    wk = w1p.tile([128, HB], F32R, tag=f"w1_{k}")
                    w1t.append(wk)
                halves = 2 if hb == 0 else 1
                for h in range(halves):
                    cs = slice(h * (HB // halves), (h + 1) * (HB // halves))
                    for k in range(KT):
                        nc.sync.dma_start(
                            w1t[k][:, cs],
                            w1_d[k * 128:(k + 1) * 128,
                                 hb * HB + cs.start:hb * HB + cs.stop],
                        )
                if hb == 0:
                    # remaining xt chunks stream behind the first weight block
                    for ch in range(1, NCH):
                        for k in range(KT):
                            sl = slice(ch * CHUNK, (ch + 1) * CHUNK)
                            nc.sync.dma_start(
                                xt[k][:, sl], xt_d[k * 128:(k + 1) * 128, sl]
                            )
                w2t = []
                for k in range(HKT):
                    wk = w2p.tile([128, D_OUT], F32R, tag=f"w2_{k}")
                    r0 = hb * HB + k * 128
                    nc.sync.dma_start(wk[:], w2_d[r0:r0 + 128, :])
                    w2t.append(wk)

                # layer 1: Ht[m] = gelu(W1_blk[:, m].T @ xt + b1)
                # chunk-outer so the first groups only need xt chunk 0
                ht = []
                for m in range(HKT):
                    ht_m = htp.tile([128, C], F32R, tag=f"ht{m}")
                    ht.append(ht_m)
                for ch in range(NCH):
                    for m in range(HKT):
                        h_col = hb * HKT + m
                        p1 = ps1.tile([128, CHUNK], F32, tag="p1")
                        for k in range(KT):
                            nc.tensor.matmul(
                                p1[:],
                                w1t[k][:, m * 128:(m + 1) * 128],
                                xt[k][:, ch * CHUNK:(ch + 1) * CHUNK],
                                start=(k == 0),
                                stop=(k == KT - 1),
                            )
                        nc.scalar.activation(
                            ht[m][:, ch * CHUNK:(ch + 1) * CHUNK],
                            p1[:],
                            AF.Gelu,
                            bias=b1c[:, h_col:h_col + 1],
                        )

                # layer 2: Y[tm] += Ht[:, tm].T @ W2_blk
                # k-outer with both n-halves per k: consecutive matmuls share
                # the stationary operand, hiding the f32r LDWEIGHTS stream.
                for tm in range(NT):
                    p2 = []
                    for n in range(2):
                        p2n = ps2.tile([128, 512], F32, tag=f"p2{n}")
                        p2.append(p2n)
                    for k in range(HKT):
                        lhs = ht[k][:, tm * 128:(tm + 1) * 128]
                        for n in range(2):
                            nc.tensor.matmul(
                                p2[n][:],
                                lhs,
                                w2t[k][:, n * 512:(n + 1) * 512],
                                start=(k == 0),
                                stop=(k == HKT - 1),
                            )
                    for n in range(2):
                        dst = y_acc[tm][:, n * 512:(n + 1) * 512]
                        if hb == 0:
                            # fold the b2 bias into the first partial sum
                            nc.vector.tensor_add(
                                dst, p2[n][:], b2b[:, n * 512:(n + 1) * 512]
                            )
                        else:
                            nc.vector.tensor_add(dst, dst, p2[n][:])

            # ---- epilogue: * gate (on ACT, overlapping DVE's last adds) ----
            for tm in range(NT):
                nc.scalar.activation(
                    y_acc[tm][:], y_acc[tm][:], AF.Identity,
                    scale=gt[:, tm:tm + 1],
                )
                nc.sync.dma_start(y_d[tm * 128:(tm + 1) * 128, :], y_acc[tm][:])

    nc.compile()
    return nc


def kernel(x, W1, b1, W2, b2, Wg, bg):
    global LAST_EXEC_NS, LAST_PROFILE
    from concourse.bass_utils import run_bass_kernel_spmd

    x = np.ascontiguousarray(np.asarray(x, dtype=np.float32))
    W1 = np.asarray(W1, dtype=np.float32)
    b1 = np.asarray(b1, dtype=np.float32)
    W2 = np.asarray(W2, dtype=np.float32)
    b2 = np.asarray(b2, dtype=np.float32)
    Wg = np.asarray(Wg, dtype=np.float32)
    bg = np.asarray(bg, dtype=np.float32)

    B, S, d = x.shape
    T = B * S
    xf = x.reshape(T, d)

    # ---- host gating + routing (replicated gating linear) ----
    logits = xf @ Wg + bg                                    # [T, E] f32
    top_idx = np.argsort(-logits, axis=1, kind="stable")[:, :TOP_K].astype(np.int32)
    vals = np.take_along_axis(logits, top_idx, axis=1)
    e_ = np.exp(vals - vals.max(axis=1, keepdims=True))
    w_topk = (e_ / e_.sum(axis=1, keepdims=True)).astype(np.float32)  # [T, k]

    # per-expert token lists ("all-to-all" dispatch)
    tok_lists, gate_lists = [], []
    for e in range(N_EXPERTS):
        sel = top_idx == e                                   # [T, k]
        toks = np.nonzero(sel.any(axis=1))[0]
        kk = np.where(sel[toks, 0], 0, 1)
        tok_lists.append(toks)
        gate_lists.append(w_topk[toks, kk])

    # SBUF sizing caps the per-launch expert capacity; extreme imbalance
    # (not seen with these shapes) falls back to multiple launches.
    CAP = 1536
    max_cnt = max(len(t) for t in tok_lists)
    n_rounds = max(1, -(-max_cnt // CAP))
    lcm = int(np.lcm(128, CHUNK))
    per_round = -(-max_cnt // n_rounds)
    C = int(max(lcm, ((per_round + lcm - 1) // lcm) * lcm))

    nc = _build_bass(C)
    final = np.zeros((T, D_OUT), np.float32)

    for r in range(n_rounds):
        in_maps = []
        r_toks = []
        for e in range(N_EXPERTS):
            toks = tok_lists[e][r * C:(r + 1) * C]
            gates = gate_lists[e][r * C:(r + 1) * C]
            r_toks.append(toks)
            cnt = len(toks)
            xt = np.zeros((D_IN, C), np.float32)
            if cnt:
                xt[:, :cnt] = xf[toks].T
            g = np.zeros(C, np.float32)
            g[:cnt] = gates
            in_maps.append({
                "xt": xt,
                "w1": np.ascontiguousarray(W1[e]),
                "w2": np.ascontiguousarray(W2[e]),
                "b1c": np.ascontiguousarray(b1[e].reshape(D_HID // 128, 128).T),
                "b2b": np.tile(b2[e], (128, 1)),
                "g": np.ascontiguousarray(g.reshape(C // 128, 128).T),
            })

        try:
            res = run_bass_kernel_spmd(nc, in_maps, list(range(N_CORES)), trace=TRACE)
        except ModuleNotFoundError:
            # NTFF profiling hook unavailable in this container; run untraced.
            res = run_bass_kernel_spmd(nc, in_maps, list(range(N_CORES)), trace=False)
        LAST_EXEC_NS = res.exec_time_ns
        LAST_PROFILE = res.profile_json

        # ---- reverse all-to-all: scatter-add gate-scaled expert outputs ----
        for e in range(N_EXPERTS):
            toks = r_toks[e]
            if len(toks):
                final[toks] += res.results[e]["y"][:len(toks)]

    return (
        final.reshape(B, S, D_OUT),
        logits.reshape(B, S, N_EXPERTS).astype(np.float32),
        top_idx.reshape(B, S, TOP_K),
    )


# revision 30
# speedup vs baseline: 1.0218x; 1.0218x over previous
"""MoE layer (1024 -> 4096 gelu -> 1024, E=8 experts, top-2) on 8 TRN2 cores.

Strategy (expert parallelism, per the sharding hint):
  - Host computes the gating linear + top-2 routing (67 MFLOP, 0.02% of the
    total work) and builds one token batch per expert ("all-to-all" dispatch).
  - Core e holds expert e's weights and runs the FFN for the tokens routed to
    it: y = gate * (gelu(x @ W1 + b1) @ W2 + b2), with the gate weight folded
    into the device epilogue.
  - Host scatter-adds the two expert outputs per token back together
    (the reverse all-to-all / unshard step).

Device kernel (per core, SPMD over 8 cores):
  - Activations live transposed (xt = X^T) so both matmul layers use natural
    weight layouts with the contraction on the partition axis.
  - W1/W2 are streamed from HBM in 8 h-blocks of 512 (32 MB per core total,
    double buffered) so weight traffic is exactly one pass.
  - Matmuls run in float32r (full-rate fp32 path on the PE, ~1e-4 precision
    vs ~3e-3 for bf16).
  - Layer 1 writes Ht = gelu(W1_blk^T @ xt + b1) [h x C] via the ACT engine;
    layer 2 accumulates Y += Ht^T @ W2_blk in PSUM over the block's 4 k-tiles
    and folds the partial into an SBUF accumulator on the DVE engine.
  - b2 is folded into the first partial sum; the epilogue scales rows by
    the gate weight and streams the result out.
"""
import numpy as np

D_IN = 1024
D_HID = 4096
D_OUT = 1024
N_EXPERTS = 8
TOP_K = 2
N_CORES = 8

HB = 512                      # h-block size streamed per iteration
N_HB = D_HID // HB            # 8 blocks
KT = D_IN // 128              # 8 contraction tiles for layer 1
HKT = HB // 128               # 4 contraction tiles per block for layer 2
CHUNK = 384                   # layer-1 moving-dim chunk (<=512, >=256 for f32r rate)

TRACE = False                 # set by test.py to collect a HW profile
LAST_EXEC_NS = None
LAST_PROFILE = None


def _build_bass(C):
    import concourse.bacc as bacc
    import concourse.tile as tile
    import concourse.mybir as mybir

    F32 = mybir.dt.float32
    F32R = mybir.dt.float32r
    AF = mybir.ActivationFunctionType

    NT = C // 128             # token tiles
    NCH = C // CHUNK          # layer-1 chunks

    nc = bacc.Bacc("TRN2", target_bir_lowering=False, debug=False)
    xt_d = nc.dram_tensor("xt", [D_IN, C], F32R, kind="ExternalInput").ap()
    w1_d = nc.dram_tensor("w1", [D_IN, D_HID], F32R, kind="ExternalInput").ap()
    w2_d = nc.dram_tensor("w2", [D_HID, D_OUT], F32R, kind="ExternalInput").ap()
    b1_d = nc.dram_tensor("b1c", [128, D_HID // 128], F32, kind="ExternalInput").ap()
    b2_d = nc.dram_tensor("b2b", [128, D_OUT], F32, kind="ExternalInput").ap()
    g_d = nc.dram_tensor("g", [128, NT], F32, kind="ExternalInput").ap()
    y_d = nc.dram_tensor("y", [C, D_OUT], F32, kind="ExternalOutput").ap()

    with tile.TileContext(nc) as tc:
        with (
            tc.tile_pool(name="persist", bufs=1) as persist,
            tc.tile_pool(name="w1p", bufs=2) as w1p,
            tc.tile_pool(name="w2p", bufs=2) as w2p,
            tc.tile_pool(name="htp", bufs=2) as htp,
            tc.tile_pool(name="ps1", bufs=3, space="PSUM") as ps1,
            tc.tile_pool(name="ps2", bufs=2, space="PSUM") as ps2,
            tc.tile_pool(name="ps_w", bufs=1, space="PSUM") as ps_w,
        ):
            # ---- PE warmup: keep the PE busy (and HAM un-throttled) while
            # the input DMAs stream in.  Zero matmuls into a scratch bank.
            warm = persist.tile([128, 512], mybir.dt.bfloat16, tag="warm")
            nc.gpsimd.memset(warm[:], 0.0)
            wps = ps_w.tile([128, 512], F32, tag="wps")
            for _ in range(40):
                nc.tensor.matmul(wps[:], warm[:, 0:128], warm[:], start=True, stop=True)

            # ---- persistent loads ----
            # xt loads are split by layer-1 chunk so the first matmul group
            # only waits for 1/NCH of the activations (chunk 0 of every
            # k-tile), letting the PE start while the rest streams in.
            xt = []
            for k in range(KT):
                xk = persist.tile([128, C], F32R, tag=f"xt{k}")
                xt.append(xk)
            for k in range(KT):
                nc.sync.dma_start(
                    xt[k][:, 0:CHUNK], xt_d[k * 128:(k + 1) * 128, 0:CHUNK]
                )
            b1c = persist.tile([128, D_HID // 128], F32, tag="b1c")
            nc.sync.dma_start(b1c[:], b1_d[:])
            b2b = persist.tile([128, D_OUT], F32, tag="b2b")
            nc.sync.dma_start(b2b[:], b2_d[:])
            gt = persist.tile([128, NT], F32, tag="g")
            nc.sync.dma_start(gt[:], g_d[:])
            y_acc = []
            for tm in range(NT):
                y_tm = persist.tile([128, D_OUT], F32, tag=f"y{tm}")
                y_acc.append(y_tm)

            # ---- h-blocked FFN ----
            for hb in range(N_HB):
                # stream this block's weights (full-BW HWDGE, f32r-direct)
                w1t = []
                for k in range(KT):
                    wk = w1p.tile([128, HB], F32R, tag=f"w1_{k}")
                    nc.sync.dma_start(
                        wk[:], w1_d[k * 128:(k + 1) * 128, hb * HB:(hb + 1) * HB]
                    )
                    w1t.append(wk)
                if hb == 0:
                    # remaining xt chunks stream behind the first weight block
                    for ch in range(1, NCH):
                        for k in range(KT):
                            sl = slice(ch * CHUNK, (ch + 1) * CHUNK)
                            nc.sync.dma_start(
                                xt[k][:, sl], xt_d[k * 128:(k + 1) * 128, sl]
                            )
                w2t = []
                for k in range(HKT):
                    wk = w2p.tile([128, D_OUT], F32R, tag=f"w2_{k}")
                    r0 = hb * HB + k * 128
                    nc.sync.dma_start(wk[:], w2_d[r0:r0 + 128, :])
                    w2t.append(wk)

                # layer 1: Ht[m] = gelu(W1_blk[:, m].T @ xt + b1)
                # chunk-outer so the first groups only need xt chunk 0
                ht = []
                for m in range(HKT):
                    ht_m = htp.tile([128, C], F32R, tag=f"ht{m}")
                    ht.append(ht_m)
                for ch in range(NCH):
                    for m in range(HKT):
                        h_col = hb * HKT + m
                        p1 = ps1.tile([128, CHUNK], F32, tag="p1")
                        for k in range(KT):
                            nc.tensor.matmul(
                                p1[:],
                                w1t[k][:, m * 128:(m + 1) * 128],
                                xt[k][:, ch * CHUNK:(ch + 1) * CHUNK],
                                start=(k == 0),
                                stop=(k == KT - 1),
                            )
                        nc.scalar.activation(
                            ht[m][:, ch * CHUNK:(ch + 1) * CHUNK],
                            p1[:],
                            AF.Gelu,
                            bias=b1c[:, h_col:h_col + 1],
                        )

                # layer 2: Y[tm] += Ht[:, tm].T @ W2_blk
                # k-outer with both n-halves per k: consecutive matmuls share
                # the stationary operand, hiding the f32r LDWEIGHTS stream.
                for tm in range(NT):
                    p2 = []
                    for n in range(2):
                        p2n = ps2.tile([128, 512], F32, tag=f"p2{n}")
                        p2.append(p2n)
                    for k in range(HKT):
                        lhs = ht[k][:, tm * 128:(tm + 1) * 128]
                        for n in range(2):
                            nc.tensor.matmul(
                                p2[n][:],
                                lhs,
                                w2t[k][:, n * 512:(n + 1) * 512],
                                start=(k == 0),
                                stop=(k == HKT - 1),
                            )
                    for n in range(2):
                        dst = y_acc[tm][:, n * 512:(n + 1) * 512]
                        if hb == 0:
                            # fold the b2 bias into the first partial sum
                            nc.vector.tensor_add(
                                dst, p2[n][:], b2b[:, n * 512:(n + 1) * 512]
                            )
                        else:
                            nc.vector.tensor_add(dst, dst, p2[n][:])

            # ---- epilogue: * gate (on ACT, overlapping DVE's last adds) ----
            for tm in range(NT):
                nc.scalar.activation(
                    y_acc[tm][:], y_acc[tm][:], AF.Identity,
                    scale=gt[:, tm:tm + 1],
                )
                nc.sync.dma_start(y_d[tm * 128:(tm + 1) * 128, :], y_acc[tm][:])

    nc.compile()
    return nc


def kernel(x, W1, b1, W2, b2, Wg, bg):
    global LAST_EXEC_NS, LAST_PROFILE
    from concourse.bass_utils import run_bass_kernel_spmd

    x = np.ascontiguousarray(np.asarray(x, dtype=np.float32))
    W1 = np.asarray(W1, dtype=np.float32)
    b1 = np.asarray(b1, dtype=np.float32)
    W2 = np.asarray(W2, dtype=np.float32)
    b2 = np.asarray(b2, dtype=np.float32)
    Wg = np.asarray(Wg, dtype=np.float32)
    bg = np.asarray(bg, dtype=np.float32)

    B, S, d = x.shape
    T = B * S
    xf = x.reshape(T, d)

    # ---- host gating + routing (replicated gating linear) ----
    logits = xf @ Wg + bg                                    # [T, E] f32
    top_idx = np.argsort(-logits, axis=1, kind="stable")[:, :TOP_K].astype(np.int32)
    vals = np.take_along_axis(logits, top_idx, axis=1)
    e_ = np.exp(vals - vals.max(axis=1, keepdims=True))
    w_topk = (e_ / e_.sum(axis=1, keepdims=True)).astype(np.float32)  # [T, k]

    # per-expert token lists ("all-to-all" dispatch)
    tok_lists, gate_lists = [], []
    for e in range(N_EXPERTS):
        sel = top_idx == e                                   # [T, k]
        toks = np.nonzero(sel.any(axis=1))[0]
        kk = np.where(sel[toks, 0], 0, 1)
        tok_lists.append(toks)
        gate_lists.append(w_topk[toks, kk])

    # SBUF sizing caps the per-launch expert capacity; extreme imbalance
    # (not seen with these shapes) falls back to multiple launches.
    CAP = 1536
    max_cnt = max(len(t) for t in tok_lists)
    n_rounds = max(1, -(-max_cnt // CAP))
    lcm = int(np.lcm(128, CHUNK))
    per_round = -(-max_cnt // n_rounds)
    C = int(max(lcm, ((per_round + lcm - 1) // lcm) * lcm))

    nc = _build_bass(C)
    final = np.zeros((T, D_OUT), np.float32)

    for r in range(n_rounds):
        in_maps = []
        r_toks = []
        for e in range(N_EXPERTS):
            toks = tok_lists[e][r * C:(r + 1) * C]
            gates = gate_lists[e][r * C:(r + 1) * C]
            r_toks.append(toks)
            cnt = len(toks)
            xt = np.zeros((D_IN, C), np.float32)
            if cnt:
                xt[:, :cnt] = xf[toks].T
            g = np.zeros(C, np.float32)
            g[:cnt] = gates
            in_maps.append({
                "xt": xt,
                "w1": np.ascontiguousarray(W1[e]),
                "w2": np.ascontiguousarray(W2[e]),
                "b1c": np.ascontiguousarray(b1[e].reshape(D_HID // 128, 128).T),
                "b2b": np.tile(b2[e], (128, 1)),
                "g": np.ascontiguousarray(g.reshape(C // 128, 128).T),
            })

        try:
            res = run_bass_kernel_spmd(nc, in_maps, list(range(N_CORES)), trace=TRACE)
        except ModuleNotFoundError:
            # NTFF profiling hook unavailable in this container; run untraced.
            res = run_bass_kernel_spmd(nc, in_maps, list(range(N_CORES)), trace=False)
        LAST_EXEC_NS = res.exec_time_ns
        LAST_PROFILE = res.profile_json

        # ---- reverse all-to-all: scatter-add gate-scaled expert outputs ----
        for e in range(N_EXPERTS):
            toks = r_toks[e]
            if len(toks):
                final[toks] += res.results[e]["y"][:len(toks)]

    return (
        final.reshape(B, S, D_OUT),
        logits.reshape(B, S, N_EXPERTS).astype(np.float32),
        top_idx.reshape(B, S, TOP_K),
    )
